# revision 5
# baseline (speedup 1.0000x reference)
"""BitLinear forward on 8 Trainium2 NeuronCores.

out = (x_q @ w_q) * (beta * gamma)
  a      = mean(weight);  w_q = sign(weight - a)
  gamma  = max|x| per row; x_q = clip(x/(gamma+eps), -(1-eps), 1-eps)
  beta   = max|weight|

Sharding: data-parallel over rows of x (N=32768 -> 4096 rows/core),
weight (1024x1024) replicated; per-core scalar stats are computed
redundantly so no collectives are needed.

Kernel math note: since QB == 1, (x_q @ w_q)*beta*gamma equals
(x @ w_q) * beta * gamma/(gamma+eps) up to the +-(1-eps) clip.  The clip
only affects the row-max element by <=1e-5 relative, and gamma/(gamma+eps)
deviates from 1 by <= eps/gamma ~ 4e-6 -- both far below the bf16 rounding
used for the matmul (~2e-3).  So the kernel never materializes x_q or even
gamma; it feeds bf16(x) to the tensor engine and multiplies the output by
the scalar beta.

Layout note: the contraction dimension may be distributed over SBUF
partitions in ANY fixed permutation as long as x^T and w_q use the same
one.  This kernel loads w as [128, 8, 1024] with partition p holding the
8 consecutive rows 8p..8p+7 (32 KiB contiguous per partition => large
DMA packets => the 4 MiB load runs near HBM rate instead of the ~150
GB/s small-packet rate).  Matmul chunk r then contracts the 128 features
{8p + r}; the bf16 cast writes x de-interleaved ([128, 8, 128], feature
f at [q, f%8, f//8]) so each transpose stationary is a contiguous slice.

Timeline (per core):
  t~2-12   weight halves on the two HWDGE queues at full rate; x tiles
           0-1 trickle on the SWDGE queues; PE transposes them.
           Remaining x is gated behind the weight DMA (a dummy gpsimd
           copy depending on the second half) so the weight load - which
           gates every matmul through mean->sign - is never starved.
  t~12-14  per-chunk row sums (ACT accum_out + DVE reduces, mostly
           hidden under the DMA), mean via ones[128,128] matmul,
           8 PE warm transposes to re-ramp the HAM-throttled clock.
  t~14-26  signs land every ~1.07us (chunk 0 split in halves for a
           faster first unlock); chunk-major matmuls over 3
           pre-transposed tiles consume them without PE bubbles.
  steady   PE runs [T8(t+3), MM16(t)] back to back; DVE casts+evacuates,
           ACT scale-copies output halves, stores alternate between the
           sync and scalar HWDGE queues.  Last two tiles split their
           stores across both queues to cut the drain tail.
"""

import sys

import numpy as np

if "/opt/trn_rl_repo" not in sys.path:
    sys.path.insert(0, "/opt/trn_rl_repo")

N_CORES = 8
N_FEAT = 1024
N_OUT = 1024
P = 128
KC = N_FEAT // P  # 8 contraction chunks of 128
EPS = 1e-5
NTILE_SINGLE = 8  # tiles 0..7 load individually (early, for PE warm)

_NC_CACHE = {}
_PATCHED = False


def _split_multi_waits(nc, max_waits=1):
    """The walrus build in this image rejects instructions carrying more
    than one sync-wait ("Too many sync wait commands").  Tile's semaphore
    assignment attaches one wait per producer proc, so hoist surplus waits
    onto NOP carrier instructions inserted immediately before the waiting
    instruction on the same engine (waits execute before the instruction
    body, so this preserves semantics exactly)."""
    import bass_rust

    for fn in nc.m.functions:
        for blk in fn.blocks:
            insts = blk.instructions  # live list
            i = 0
            while i < len(insts):
                ins = insts[i]
                si = getattr(ins, "sync_info", None)
                if si is None:
                    i += 1
                    continue
                waits = list(si.on_wait)
                if len(waits) <= max_waits:
                    i += 1
                    continue
                keep = waits[:max_waits]
                surplus = waits[max_waits:]
                si.on_wait = keep
                carriers = []
                cur_list = nc.cur_bb.bb.instructions
                for j in range(0, len(surplus), max_waits):
                    nop = nc.engines[ins.engine].nop(nofuse=True)
                    nop.ins.sync_info = bass_rust.SyncInfo(
                        on_wait=surplus[j : j + max_waits], on_update=[]
                    )
                    popped = cur_list.pop()
                    assert popped is nop.ins
                    carriers.append(nop.ins)
                for k, c in enumerate(carriers):
                    insts.insert(i + k, c)
                i += len(carriers) + 1


def _patch_tile_drain():
    global _PATCHED
    if _PATCHED:
        return
    _PATCHED = True
    import concourse.tile as tile

    orig = tile.TileContext._drain_and_barrier

    def patched(self, tick_clock, wait_clock):
        orig(self, tick_clock, wait_clock)
        _split_multi_waits(self.nc)

    tile.TileContext._drain_and_barrier = patched


def _build_nc(rows_per_core: int):
    import concourse.bass as bass
    import concourse.mybir as mybir
    import concourse.tile as tile

    _patch_tile_drain()

    f32 = mybir.dt.float32
    bf16 = mybir.dt.bfloat16
    R = rows_per_core
    assert R % (4 * P) == 0
    T = R // P                      # 32 tiles of 128 rows
    NG = (T - NTILE_SINGLE) // 4    # 4-tile groups for tiles 8..T-1
    GOFF = NTILE_SINGLE // 4        # group index offset (rows 0..1023 are tiles)

    nc = bass.Bass(
        "TRN2", target_bir_lowering=False, debug=False
    )
    x_h = nc.declare_dram_parameter("x", [R, N_FEAT], f32, isOutput=False)
    w_h = nc.declare_dram_parameter("weight", [N_FEAT, N_OUT], f32, isOutput=False)
    i_h = nc.declare_dram_parameter("ident", [P, P], bf16, isOutput=False)
    o_h = nc.declare_dram_parameter("out", [R, N_OUT], f32, isOutput=True)

    # weight[8p + r, n] -> [p, r, n]: 32 KiB contiguous per partition
    w_ap = w_h[:, :].rearrange("(p r) n -> p r n", r=KC)
    # x single tiles: [t, q, c, k], feature f = 8c + k, 4 KiB lines
    xt_ap = x_h[:, :].rearrange("(t q) (c k) -> t q c k", q=P, k=KC)
    # x groups of 512 rows: row = 512g + 4q + r, 16 KiB lines
    xg_ap = x_h[:, :].rearrange("(g q r) (c k) -> g q r c k", q=P, r=4, k=KC)
    ot_ap = o_h[:, :].rearrange("(t q) n -> t q n", q=P)
    og_ap = o_h[:, :].rearrange("(g q r) n -> g r q n", q=P, r=4)

    with tile.TileContext(nc) as tc:
        with (
            tc.tile_pool(name="wpool", bufs=1) as wpool,
            tc.tile_pool(name="x1pool", bufs=NTILE_SINGLE) as x1pool,
            tc.tile_pool(name="xgpool", bufs=3) as xgpool,
            tc.tile_pool(name="bpool", bufs=4) as bpool,
            tc.tile_pool(name="tpool", bufs=7) as tpool,
            tc.tile_pool(name="opool", bufs=4) as opool,
            tc.tile_pool(name="pspool", bufs=3, space="PSUM") as pspool,
            tc.tile_pool(name="ps1pool", bufs=2, space="PSUM") as ps1pool,
        ):
            # ---- persistent weight-side tiles ----
            w32 = wpool.tile([P, KC, N_OUT], f32, tag="w32")
            wq = wpool.tile([P, KC, N_OUT], bf16, tag="wq")
            wsum = wpool.tile([P, KC], f32, tag="wsum")
            bmax = wpool.tile([P, KC], f32, tag="bmax")
            bmax1 = wpool.tile([P, 1], f32, tag="bmax1")
            pack2 = wpool.tile([1, 2], f32, tag="pack2")
            ones1 = wpool.tile([1, P], f32, tag="ones1")
            ssum = wpool.tile([P, 1], f32, tag="ssum")
            ones128 = wpool.tile([P, P], f32, tag="ones128")
            stats = wpool.tile([P, 2], f32, tag="stats")
            gate = wpool.tile([P, 1], f32, tag="gate")
            ident = wpool.tile([P, P], bf16, tag="ident")

            neg_a = stats[:, 0:1]
            beta = stats[:, 1:2]

            # ---- doorbells first: weight halves own the HWDGE queues ----
            nc.sync.dma_start(out=w32[:, 0 : KC // 2, :], in_=w_ap[:, 0 : KC // 2, :])
            nc.scalar.dma_start(out=w32[:, KC // 2 :, :], in_=w_ap[:, KC // 2 :, :])
            nc.sync.dma_start(out=ident, in_=i_h[:, :])
            nc.vector.memset(ones128, 1.0)
            nc.vector.memset(ones1, 1.0)

            def emit_x_chain(t):
                """cast + 8 PE transposes + evacuation for one 128-row tile."""
                if t < NTILE_SINGLE:
                    x32 = x1pool.tile([P, P, KC], f32, tag="x32")
                    nc.gpsimd.dma_start(out=x32, in_=xt_ap[t, :, :, :])
                    src = x32[:, :, :]
                else:
                    g, r = divmod(t - NTILE_SINGLE, 4)
                    if r == 0:
                        xg = xgpool.tile([P, 4, P, KC], f32, tag="xg", name=f"xg{g}")
                        nc.gpsimd.dma_start(
                            out=xg, in_=xg_ap[g + GOFF, :, :, :, :]
                        )
                        emit_x_chain.cur_group = xg
                    src = emit_x_chain.cur_group[:, r, :, :]
                # de-interleaving cast: xb[q, k, c] = x[row_q, 8c + k]
                xb = bpool.tile([P, KC, P], bf16, tag="xb")
                nc.vector.tensor_copy(out=xb[:, :, :].transpose([0, 2, 1]), in_=src)
                xTps = ps1pool.tile([P, KC, P], bf16, tag="xTps")
                for k in range(KC):
                    nc.tensor.transpose(xTps[:, k, :], xb[:, k, :], ident)
                xT = tpool.tile([P, KC, P], bf16, tag="xT")
                nc.vector.tensor_copy(out=xT, in_=xTps)
                return xT

            def store_out(t, o, split):
                """DMA the finished [128, 1024] output tile."""
                if t < NTILE_SINGLE:
                    dst = ot_ap[t, :, :]
                else:
                    g, r = divmod(t - NTILE_SINGLE, 4)
                    dst = og_ap[g + GOFF, r, :, :]
                if split:
                    nc.sync.dma_start(out=dst[:, 0:512], in_=o[:, 0:512])
                    nc.scalar.dma_start(out=dst[:, 512:1024], in_=o[:, 512:1024])
                elif t % 2 == 0:
                    nc.sync.dma_start(out=dst, in_=o)
                else:
                    nc.scalar.dma_start(out=dst, in_=o)

            # ---- x tiles 0-1 early (PE warm food), chains emitted now ----
            xT_list = {}
            xT_list[0] = emit_x_chain(0)
            xT_list[1] = emit_x_chain(1)

            # ---- row sums, half A (hidden under half-B DMA) ----
            for r in (0, 1):
                nc.scalar.activation(
                    out=wq[:, r, :], in_=w32[:, r, :],
                    func=mybir.ActivationFunctionType.Copy,
                    bias=0.0, scale=1.0,
                    accum_out=wsum[:, r : r + 1],
                )
            for r in (2, 3):
                nc.vector.tensor_reduce(
                    wsum[:, r : r + 1], w32[:, r, :],
                    axis=mybir.AxisListType.X, op=mybir.AluOpType.add,
                )

            # ---- gate the bulk of x behind the weight DMA so the weight
            # load (which gates everything through mean->sign) gets the
            # full DMA bandwidth ----
            nc.gpsimd.tensor_copy(out=gate, in_=w32[:, KC - 1, 0:1])
            for t in range(2, NTILE_SINGLE):
                xT_list[t] = emit_x_chain(t)

            # ---- row sums, half B + mean ----
            for r in (4, 5, 6):
                nc.vector.tensor_reduce(
                    wsum[:, r : r + 1], w32[:, r, :],
                    axis=mybir.AxisListType.X, op=mybir.AluOpType.add,
                )
            nc.scalar.activation(
                out=wq[:, 7, :], in_=w32[:, 7, :],
                func=mybir.ActivationFunctionType.Copy,
                bias=0.0, scale=1.0,
                accum_out=wsum[:, 7:8],
            )
            nc.vector.tensor_reduce(
                ssum, wsum, axis=mybir.AxisListType.X, op=mybir.AluOpType.add
            )
            # re-warm the PE clock right before the first real matmuls
            warm_ps = ps1pool.tile([P, P], bf16, tag="xTps")
            for _ in range(8):
                nc.tensor.transpose(warm_ps, ident, ident)
            # ones[128,128] matmul: reduces ssum across partitions AND
            # replicates the total to all 128 partitions in one shot
            na_ps = ps1pool.tile([P, 1], f32, tag="xTps")
            nc.tensor.matmul(na_ps, ones128, ssum, start=True, stop=True)
            nc.vector.tensor_scalar_mul(
                neg_a, na_ps, -1.0 / float(N_FEAT * N_OUT)
            )

            # ---- signs; chunk 0 in halves for a faster first unlock ----
            nc.scalar.activation(
                out=wq[:, 0, 0:512], in_=w32[:, 0, 0:512],
                func=mybir.ActivationFunctionType.Sign, bias=neg_a, scale=1.0,
            )
            nc.scalar.activation(
                out=wq[:, 0, 512:1024], in_=w32[:, 0, 512:1024],
                func=mybir.ActivationFunctionType.Sign, bias=neg_a, scale=1.0,
            )
            for r in range(1, KC):
                nc.scalar.activation(
                    out=wq[:, r, :], in_=w32[:, r, :],
                    func=mybir.ActivationFunctionType.Sign, bias=neg_a, scale=1.0,
                )

            # ---- warm matmuls: chunk-major over tiles 0-2 so each
            # arriving sign feeds 3 tiles (1.28us PE work per 1.07us sign) ----
            NW = 3
            ps_w = [
                pspool.tile([P, N_OUT], f32, tag="ps", name=f"ps_w{i}")
                for i in range(NW)
            ]
            for k in range(KC):
                for h in range(2):
                    for ti in range(NW):
                        nc.tensor.matmul(
                            ps_w[ti][:, h * 512 : (h + 1) * 512],
                            xT_list[ti][:, k, :],
                            wq[:, k, h * 512 : (h + 1) * 512],
                            start=(k == 0),
                            stop=(k == KC - 1),
                        )

            # ---- beta = max|w| (needed only by the first output copy) ----
            for r in range(KC):
                nc.vector.tensor_reduce(
                    bmax[:, r : r + 1], w32[:, r, :],
                    axis=mybir.AxisListType.X, op=mybir.AluOpType.max,
                    apply_absolute_value=True,
                )
            nc.vector.tensor_reduce(
                bmax1, bmax, axis=mybir.AxisListType.X, op=mybir.AluOpType.max
            )
            nc.gpsimd.tensor_reduce(
                pack2[:, 1:2], bmax1, axis=mybir.AxisListType.C,
                op=mybir.AluOpType.max,
            )
            b_ps = ps1pool.tile([P, 1], f32, tag="xTps")
            nc.tensor.matmul(b_ps, ones1, pack2[:, 1:2], start=True, stop=True)
            nc.vector.tensor_copy(out=beta, in_=b_ps)

            # ---- warm outputs ----
            for ti in range(NW):
                o = opool.tile([P, N_OUT], f32, tag="o", name=f"o_w{ti}")
                for h in range(2):
                    nc.scalar.activation(
                        out=o[:, h * 512 : (h + 1) * 512],
                        in_=ps_w[ti][:, h * 512 : (h + 1) * 512],
                        func=mybir.ActivationFunctionType.Copy,
                        bias=0.0, scale=beta,
                    )
                store_out(ti, o, split=False)

            # ---- steady loop: PE stream is [T8(t+3), MM16(t)] ----
            for t in range(NW, T):
                if t + 3 < T:
                    xT_list[t + 3] = emit_x_chain(t + 3)
                xT = xT_list.pop(t)
                ps = pspool.tile([P, N_OUT], f32, tag="ps")
                for k in range(KC):
                    for h in range(2):
                        nc.tensor.matmul(
                            ps[:, h * 512 : (h + 1) * 512],
                            xT[:, k, :],
                            wq[:, k, h * 512 : (h + 1) * 512],
                            start=(k == 0),
                            stop=(k == KC - 1),
                        )
                o = opool.tile([P, N_OUT], f32, tag="o")
                for h in range(2):
                    nc.scalar.activation(
                        out=o[:, h * 512 : (h + 1) * 512],
                        in_=ps[:, h * 512 : (h + 1) * 512],
                        func=mybir.ActivationFunctionType.Copy,
                        bias=0.0, scale=beta,
                    )
                store_out(t, o, split=(t >= T - 2))

    return nc


def _get_nc(rows_per_core: int):
    if rows_per_core not in _NC_CACHE:
        _NC_CACHE[rows_per_core] = _build_nc(rows_per_core)
    return _NC_CACHE[rows_per_core]


def run(x, weight, trace=False, trace_cores=None):
    """Run on 8 cores; returns (out, BassKernelResults)."""
    from concourse.bass_utils import run_bass_kernel_spmd

    import ml_dtypes

    x = np.ascontiguousarray(np.asarray(x, dtype=np.float32))
    weight = np.ascontiguousarray(np.asarray(weight, dtype=np.float32))
    ident = np.eye(P, dtype=ml_dtypes.bfloat16)
    n = x.shape[0]
    assert n % N_CORES == 0
    rpc = n // N_CORES
    nc = _get_nc(rpc)
    in_maps = [
        {"x": x[i * rpc : (i + 1) * rpc], "weight": weight, "ident": ident}
        for i in range(N_CORES)
    ]
    kwargs = {}
    if trace:
        kwargs["trace"] = True
        if trace_cores is not None:
            kwargs["trace_cores"] = trace_cores
    res = run_bass_kernel_spmd(nc, in_maps, core_ids=list(range(N_CORES)), **kwargs)
    out = np.concatenate([r["out"] for r in res.results], axis=0)
    return out, res


def kernel(x, weight):
    out, _ = run(x, weight)
    return out


# revision 6
# speedup vs baseline: 1.0736x; 1.0736x over previous
"""BitLinear forward on 8 Trainium2 NeuronCores.

out = (x_q @ w_q) * (beta * gamma)
  a      = mean(weight);  w_q = sign(weight - a)
  gamma  = max|x| per row; x_q = clip(x/(gamma+eps), -(1-eps), 1-eps)
  beta   = max|weight|

Sharding: data-parallel over rows of x (N=32768 -> 4096 rows/core),
weight (1024x1024) replicated; per-core scalar stats are computed
redundantly so no collectives are needed.

Kernel math note: since QB == 1, (x_q @ w_q)*beta*gamma equals
(x @ w_q) * beta * gamma/(gamma+eps) up to the +-(1-eps) clip.  The clip
only affects the row-max element by <=1e-5 relative, and gamma/(gamma+eps)
deviates from 1 by <= eps/gamma ~ 4e-6 -- both far below the bf16 rounding
used for the matmul (~2e-3).  So the kernel never materializes x_q or even
gamma; it feeds bf16(x) to the tensor engine and multiplies the output by
the scalar beta.

Layout note: the contraction dimension may be distributed over SBUF
partitions in ANY fixed permutation as long as x^T and w_q use the same
one.  This kernel loads w as [128, 8, 1024] with partition p holding the
8 consecutive rows 8p..8p+7 (32 KiB contiguous per partition => large
DMA packets => the 4 MiB load runs near HBM rate instead of the ~150
GB/s small-packet rate).  Matmul chunk r then contracts the 128 features
{8p + r}; the bf16 cast writes x de-interleaved ([128, 8, 128], feature
f at [q, f%8, f//8]) so each transpose stationary is a contiguous slice.

Timeline (per core):
  t~2-12   weight halves on the two HWDGE queues at full rate; x tiles
           0-1 trickle on the SWDGE queues; PE transposes them.
           Remaining x is gated behind the weight DMA (a dummy gpsimd
           copy depending on the second half) so the weight load - which
           gates every matmul through mean->sign - is never starved.
  t~12-14  per-chunk row sums (ACT accum_out + DVE reduces, mostly
           hidden under the DMA), mean via ones[128,128] matmul,
           8 PE warm transposes to re-ramp the HAM-throttled clock.
  t~14-26  signs land every ~1.07us (chunk 0 split in halves for a
           faster first unlock); chunk-major matmuls over 3
           pre-transposed tiles consume them without PE bubbles.
  steady   PE runs [T8(t+3), MM16(t)] back to back; DVE casts+evacuates,
           ACT scale-copies output halves, stores alternate between the
           sync and scalar HWDGE queues.  Last two tiles split their
           stores across both queues to cut the drain tail.
"""

import sys

import numpy as np

if "/opt/trn_rl_repo" not in sys.path:
    sys.path.insert(0, "/opt/trn_rl_repo")

N_CORES = 8
N_FEAT = 1024
N_OUT = 1024
P = 128
KC = N_FEAT // P  # 8 contraction chunks of 128
EPS = 1e-5
NTILE_SINGLE = 8  # tiles 0..7 load individually (early, for PE warm)

_NC_CACHE = {}
_PATCHED = False


def _split_multi_waits(nc, max_waits=1):
    """The walrus build in this image rejects instructions carrying more
    than one sync-wait ("Too many sync wait commands").  Tile's semaphore
    assignment attaches one wait per producer proc, so hoist surplus waits
    onto NOP carrier instructions inserted immediately before the waiting
    instruction on the same engine (waits execute before the instruction
    body, so this preserves semantics exactly)."""
    import bass_rust

    for fn in nc.m.functions:
        for blk in fn.blocks:
            insts = blk.instructions  # live list
            i = 0
            while i < len(insts):
                ins = insts[i]
                si = getattr(ins, "sync_info", None)
                if si is None:
                    i += 1
                    continue
                waits = list(si.on_wait)
                if len(waits) <= max_waits:
                    i += 1
                    continue
                keep = waits[:max_waits]
                surplus = waits[max_waits:]
                si.on_wait = keep
                carriers = []
                cur_list = nc.cur_bb.bb.instructions
                for j in range(0, len(surplus), max_waits):
                    nop = nc.engines[ins.engine].nop(nofuse=True)
                    nop.ins.sync_info = bass_rust.SyncInfo(
                        on_wait=surplus[j : j + max_waits], on_update=[]
                    )
                    popped = cur_list.pop()
                    assert popped is nop.ins
                    carriers.append(nop.ins)
                for k, c in enumerate(carriers):
                    insts.insert(i + k, c)
                i += len(carriers) + 1


def _patch_tile_drain():
    global _PATCHED
    if _PATCHED:
        return
    _PATCHED = True
    import concourse.tile as tile

    orig = tile.TileContext._drain_and_barrier

    def patched(self, tick_clock, wait_clock):
        orig(self, tick_clock, wait_clock)
        _split_multi_waits(self.nc)

    tile.TileContext._drain_and_barrier = patched


def _build_nc(rows_per_core: int):
    import concourse.bass as bass
    import concourse.mybir as mybir
    import concourse.tile as tile

    _patch_tile_drain()

    f32 = mybir.dt.float32
    bf16 = mybir.dt.bfloat16
    R = rows_per_core
    assert R % (4 * P) == 0
    T = R // P                      # 32 tiles of 128 rows
    NG = (T - NTILE_SINGLE) // 4    # 4-tile groups for tiles 8..T-1
    GOFF = NTILE_SINGLE // 4        # group index offset (rows 0..1023 are tiles)

    nc = bass.Bass(
        "TRN2", target_bir_lowering=False, debug=False
    )
    x_h = nc.declare_dram_parameter("x", [R, N_FEAT], f32, isOutput=False)
    w_h = nc.declare_dram_parameter("weight", [N_FEAT, N_OUT], f32, isOutput=False)
    i_h = nc.declare_dram_parameter("ident", [P, P], bf16, isOutput=False)
    o_h = nc.declare_dram_parameter("out", [R, N_OUT], f32, isOutput=True)

    # weight[8p + r, n] -> [p, (r n)]: ONE contiguous 32 KiB run per
    # partition so the DGE emits big packets (multi-dim free patterns
    # fragment into per-run descriptors)
    w_ap = w_h[:, :].rearrange("(p r) n -> p (r n)", r=KC)
    # x single tiles: [t, q, n], 4 KiB contiguous lines
    xt_ap = x_h[:, :].rearrange("(t q) n -> t q n", q=P)
    # x groups of 512 rows: row = 512g + 4q + r, 16 KiB contiguous lines
    xg_ap = x_h[:, :].rearrange("(g q r) n -> g q (r n)", q=P, r=4)
    ot_ap = o_h[:, :].rearrange("(t q) n -> t q n", q=P)
    og_ap = o_h[:, :].rearrange("(g q r) n -> g r q n", q=P, r=4)

    with tile.TileContext(nc) as tc:
        with (
            tc.tile_pool(name="wpool", bufs=1) as wpool,
            tc.tile_pool(name="x1pool", bufs=NTILE_SINGLE) as x1pool,
            tc.tile_pool(name="xgpool", bufs=3) as xgpool,
            tc.tile_pool(name="bpool", bufs=4) as bpool,
            tc.tile_pool(name="tpool", bufs=7) as tpool,
            tc.tile_pool(name="opool", bufs=4) as opool,
            tc.tile_pool(name="pspool", bufs=3, space="PSUM") as pspool,
            tc.tile_pool(name="ps1pool", bufs=2, space="PSUM") as ps1pool,
        ):
            # ---- persistent weight-side tiles (flat [128, 8192]; chunk r
            # of the contraction = features {8p + r} = columns r*1024..) ----
            w32 = wpool.tile([P, KC * N_OUT], f32, tag="w32")
            wq = wpool.tile([P, KC * N_OUT], bf16, tag="wq")
            wsum = wpool.tile([P, KC], f32, tag="wsum")
            bmax = wpool.tile([P, KC], f32, tag="bmax")
            bmax1 = wpool.tile([P, 1], f32, tag="bmax1")
            pack2 = wpool.tile([1, 2], f32, tag="pack2")
            ones1 = wpool.tile([1, P], f32, tag="ones1")
            ssum = wpool.tile([P, 1], f32, tag="ssum")
            ones128 = wpool.tile([P, P], f32, tag="ones128")
            stats = wpool.tile([P, 2], f32, tag="stats")
            gate = wpool.tile([P, 1], f32, tag="gate")
            ident = wpool.tile([P, P], bf16, tag="ident")

            neg_a = stats[:, 0:1]
            beta = stats[:, 1:2]

            # ---- doorbells first: weight halves own the HWDGE queues ----
            HW = KC * N_OUT // 2
            nc.sync.dma_start(out=w32[:, 0:HW], in_=w_ap[:, 0:HW])
            nc.scalar.dma_start(out=w32[:, HW:], in_=w_ap[:, HW:])
            nc.sync.dma_start(out=ident, in_=i_h[:, :])
            nc.vector.memset(ones128, 1.0)
            nc.vector.memset(ones1, 1.0)

            def emit_x_chain(t):
                """cast + 8 PE transposes + evacuation for one 128-row tile."""
                if t < NTILE_SINGLE:
                    x32 = x1pool.tile([P, N_FEAT], f32, tag="x32")
                    nc.gpsimd.dma_start(out=x32, in_=xt_ap[t, :, :])
                    src = x32[:, :]
                else:
                    g, r = divmod(t - NTILE_SINGLE, 4)
                    if r == 0:
                        xg = xgpool.tile([P, 4 * N_FEAT], f32, tag="xg", name=f"xg{g}")
                        nc.gpsimd.dma_start(out=xg, in_=xg_ap[g + GOFF, :, :])
                        emit_x_chain.cur_group = xg
                    src = emit_x_chain.cur_group[:, r * N_FEAT : (r + 1) * N_FEAT]
                xb = bpool.tile([P, N_FEAT], bf16, tag="xb")
                nc.vector.tensor_copy(out=xb, in_=src)
                # chunk k of the contraction = features {8c + k}: stride-8
                # read in the transpose stationary matches the w layout
                xb_r = xb[:, :].rearrange("q (c k) -> q k c", k=KC)
                xTps = ps1pool.tile([P, KC, P], bf16, tag="xTps")
                for k in range(KC):
                    nc.tensor.transpose(xTps[:, k, :], xb_r[:, k, :], ident)
                xT = tpool.tile([P, KC, P], bf16, tag="xT")
                nc.vector.tensor_copy(out=xT, in_=xTps)
                return xT

            def store_out(t, o, split):
                """DMA the finished [128, 1024] output tile."""
                if t < NTILE_SINGLE:
                    dst = ot_ap[t, :, :]
                else:
                    g, r = divmod(t - NTILE_SINGLE, 4)
                    dst = og_ap[g + GOFF, r, :, :]
                if split:
                    nc.sync.dma_start(out=dst[:, 0:512], in_=o[:, 0:512])
                    nc.scalar.dma_start(out=dst[:, 512:1024], in_=o[:, 512:1024])
                elif t % 2 == 0:
                    nc.sync.dma_start(out=dst, in_=o)
                else:
                    nc.scalar.dma_start(out=dst, in_=o)

            # ---- x tiles 0-1 early (PE warm food), chains emitted now ----
            xT_list = {}
            xT_list[0] = emit_x_chain(0)
            xT_list[1] = emit_x_chain(1)

            # ---- row sums, half A (hidden under half-B DMA) ----
            def wsl(tile_, r, lo=0, hi=N_OUT):
                return tile_[:, r * N_OUT + lo : r * N_OUT + hi]

            for r in (0, 1):
                nc.scalar.activation(
                    out=wsl(wq, r), in_=wsl(w32, r),
                    func=mybir.ActivationFunctionType.Copy,
                    bias=0.0, scale=1.0,
                    accum_out=wsum[:, r : r + 1],
                )
            for r in (2, 3):
                nc.vector.tensor_reduce(
                    wsum[:, r : r + 1], wsl(w32, r),
                    axis=mybir.AxisListType.X, op=mybir.AluOpType.add,
                )

            # ---- gate the bulk of x behind the weight DMA so the weight
            # load (which gates everything through mean->sign) gets the
            # full DMA bandwidth ----
            nc.gpsimd.tensor_copy(out=gate, in_=w32[:, KC * N_OUT - 1 :])
            for t in range(2, NTILE_SINGLE):
                xT_list[t] = emit_x_chain(t)

            # ---- row sums, half B + mean ----
            for r in (4, 5, 6):
                nc.vector.tensor_reduce(
                    wsum[:, r : r + 1], wsl(w32, r),
                    axis=mybir.AxisListType.X, op=mybir.AluOpType.add,
                )
            nc.scalar.activation(
                out=wsl(wq, 7), in_=wsl(w32, 7),
                func=mybir.ActivationFunctionType.Copy,
                bias=0.0, scale=1.0,
                accum_out=wsum[:, 7:8],
            )
            nc.vector.tensor_reduce(
                ssum, wsum, axis=mybir.AxisListType.X, op=mybir.AluOpType.add
            )
            # re-warm the PE clock right before the first real matmuls
            warm_ps = ps1pool.tile([P, P], bf16, tag="xTps")
            for _ in range(8):
                nc.tensor.transpose(warm_ps, ident, ident)
            # ones[128,128] matmul: reduces ssum across partitions AND
            # replicates the total to all 128 partitions in one shot
            na_ps = ps1pool.tile([P, 1], f32, tag="xTps")
            nc.tensor.matmul(na_ps, ones128, ssum, start=True, stop=True)
            nc.vector.tensor_scalar_mul(
                neg_a, na_ps, -1.0 / float(N_FEAT * N_OUT)
            )

            # ---- signs; chunk 0 in halves for a faster first unlock ----
            nc.scalar.activation(
                out=wsl(wq, 0, 0, 512), in_=wsl(w32, 0, 0, 512),
                func=mybir.ActivationFunctionType.Sign, bias=neg_a, scale=1.0,
            )
            nc.scalar.activation(
                out=wsl(wq, 0, 512, 1024), in_=wsl(w32, 0, 512, 1024),
                func=mybir.ActivationFunctionType.Sign, bias=neg_a, scale=1.0,
            )
            for r in range(1, KC):
                nc.scalar.activation(
                    out=wsl(wq, r), in_=wsl(w32, r),
                    func=mybir.ActivationFunctionType.Sign, bias=neg_a, scale=1.0,
                )

            # ---- warm matmuls: chunk-major over tiles 0-2 so each
            # arriving sign feeds 3 tiles (1.28us PE work per 1.07us sign) ----
            NW = 3
            ps_w = [
                pspool.tile([P, N_OUT], f32, tag="ps", name=f"ps_w{i}")
                for i in range(NW)
            ]
            for k in range(KC):
                for h in range(2):
                    for ti in range(NW):
                        nc.tensor.matmul(
                            ps_w[ti][:, h * 512 : (h + 1) * 512],
                            xT_list[ti][:, k, :],
                            wsl(wq, k, h * 512, (h + 1) * 512),
                            start=(k == 0),
                            stop=(k == KC - 1),
                        )

            # ---- beta = max|w| (needed only by the first output copy) ----
            for r in range(KC):
                nc.vector.tensor_reduce(
                    bmax[:, r : r + 1], wsl(w32, r),
                    axis=mybir.AxisListType.X, op=mybir.AluOpType.max,
                    apply_absolute_value=True,
                )
            nc.vector.tensor_reduce(
                bmax1, bmax, axis=mybir.AxisListType.X, op=mybir.AluOpType.max
            )
            nc.gpsimd.tensor_reduce(
                pack2[:, 1:2], bmax1, axis=mybir.AxisListType.C,
                op=mybir.AluOpType.max,
            )
            b_ps = ps1pool.tile([P, 1], f32, tag="xTps")
            nc.tensor.matmul(b_ps, ones1, pack2[:, 1:2], start=True, stop=True)
            nc.vector.tensor_copy(out=beta, in_=b_ps)

            # ---- warm outputs ----
            for ti in range(NW):
                o = opool.tile([P, N_OUT], f32, tag="o", name=f"o_w{ti}")
                for h in range(2):
                    nc.scalar.activation(
                        out=o[:, h * 512 : (h + 1) * 512],
                        in_=ps_w[ti][:, h * 512 : (h + 1) * 512],
                        func=mybir.ActivationFunctionType.Copy,
                        bias=0.0, scale=beta,
                    )
                store_out(ti, o, split=False)

            # ---- steady loop: PE stream is [T8(t+3), MM16(t)] ----
            for t in range(NW, T):
                if t + 3 < T:
                    xT_list[t + 3] = emit_x_chain(t + 3)
                xT = xT_list.pop(t)
                ps = pspool.tile([P, N_OUT], f32, tag="ps")
                for k in range(KC):
                    for h in range(2):
                        nc.tensor.matmul(
                            ps[:, h * 512 : (h + 1) * 512],
                            xT[:, k, :],
                            wsl(wq, k, h * 512, (h + 1) * 512),
                            start=(k == 0),
                            stop=(k == KC - 1),
                        )
                o = opool.tile([P, N_OUT], f32, tag="o")
                for h in range(2):
                    nc.scalar.activation(
                        out=o[:, h * 512 : (h + 1) * 512],
                        in_=ps[:, h * 512 : (h + 1) * 512],
                        func=mybir.ActivationFunctionType.Copy,
                        bias=0.0, scale=beta,
                    )
                store_out(t, o, split=(t >= T - 2))

    return nc


def _get_nc(rows_per_core: int):
    if rows_per_core not in _NC_CACHE:
        _NC_CACHE[rows_per_core] = _build_nc(rows_per_core)
    return _NC_CACHE[rows_per_core]


def run(x, weight, trace=False, trace_cores=None):
    """Run on 8 cores; returns (out, BassKernelResults)."""
    from concourse.bass_utils import run_bass_kernel_spmd

    import ml_dtypes

    x = np.ascontiguousarray(np.asarray(x, dtype=np.float32))
    weight = np.ascontiguousarray(np.asarray(weight, dtype=np.float32))
    ident = np.eye(P, dtype=ml_dtypes.bfloat16)
    n = x.shape[0]
    assert n % N_CORES == 0
    rpc = n // N_CORES
    nc = _get_nc(rpc)
    in_maps = [
        {"x": x[i * rpc : (i + 1) * rpc], "weight": weight, "ident": ident}
        for i in range(N_CORES)
    ]
    kwargs = {}
    if trace:
        kwargs["trace"] = True
        if trace_cores is not None:
            kwargs["trace_cores"] = trace_cores
    res = run_bass_kernel_spmd(nc, in_maps, core_ids=list(range(N_CORES)), **kwargs)
    out = np.concatenate([r["out"] for r in res.results], axis=0)
    return out, res


def kernel(x, weight):
    out, _ = run(x, weight)
    return out
